# revision 58
# baseline (speedup 1.0000x reference)
"""LLaMA causal self-attention (GQA) on 8 Trainium2 NeuronCores.

Sharding: 2-way data-parallel over batch x 4-way tensor-parallel over KV
groups. Core cid handles batch b=cid//4 and KV group g=cid%4 (q heads
4g..4g+3, kv head g). Each core computes a partial output y_partial =
att_heads @ Wo_rows; the host sums the 4 partials per batch and adds bo.

Per-core pipeline (all layouts chosen so matmul contraction is on the
partition dim and softmax needs no transposes):
  P1: QKV projection (x^T chunks stationary), optional bias via K=1
      ones-row matmul, RMSNorm (ACT Square+accum, Newton-refined rsqrt),
      RoPE (elementwise, with qn_w/kn_w folded into the trig tables).
      The PE-transposes of q/k to [d, t] layout for tile tt are issued
      after tile tt+1's QKV matmuls so the in-order PE never stalls on
      the ACT/DVE norm+rope chain; PSUM->SBUF copies ride the Pool
      engine.
  P2: attention computed transposed: scoresT[k, q] = kT_j^T @ qT in
      512-column q chunks, additive causal mask on diagonal blocks, exp
      without max subtraction (RMS-normed scores are bounded, softmax is
      shift invariant), softmax denominators via an all-ones stationary
      matmul (sums land broadcast across all partitions), PV accumulated
      in PSUM over j. Accumulator PSUM is double-buffered so the DVE
      normalize (reciprocal+mul) of chunk c overlaps the matmuls of
      chunk c+1.
  P3: output projection from attT chunks, PSUM -> SBUF -> DRAM.

Matmul operands are bf16 (PE runs 2x the fp32r column rate); softmax
statistics and PSUM accumulation stay fp32.
"""

import os
from contextlib import ExitStack

import numpy as np

B, T, C = 2, 2048, 2048
H, KV = 16, 4
D = 128
HQ = H // KV        # q heads per core = 4
TB = T // 128       # 16
CB = C // 128       # 16
EPS = 1e-5
SCALE = float(np.float32(1.0) / np.sqrt(np.float32(D)))
QC = 4              # q chunks of 512 per head in P2

_CACHE = {}


def _build(dt_name, use_bias):
    import concourse.bass as bass
    import concourse.bacc as bacc
    from concourse import mybir
    from concourse.tile import TileContext

    DT = getattr(mybir.dt, dt_name)
    F32 = mybir.dt.float32
    AF = mybir.ActivationFunctionType
    ALU = mybir.AluOpType

    nc = bacc.Bacc(None, target_bir_lowering=False)
    xt = nc.dram_tensor("xt", [TB, 128, CB * 128], DT, kind="ExternalInput")
    wqkv = nc.dram_tensor("wqkv", [CB, 128, 768], DT, kind="ExternalInput")
    bqkv = nc.dram_tensor("bqkv", [1, 768], DT, kind="ExternalInput")
    trig = nc.dram_tensor("trig", [TB, 128, 8 * 64], DT, kind="ExternalInput")
    maskt = nc.dram_tensor("maskt", [128, 128], F32, kind="ExternalInput")
    cst = nc.dram_tensor("cst", [2, 128, 128], DT, kind="ExternalInput")
    wo = nc.dram_tensor("wo", [HQ, 128, C], DT, kind="ExternalInput")
    y = nc.dram_tensor("y", [T, C], DT, kind="ExternalOutput")

    def bc4(apv, n):
        # broadcast a [128, 64] AP along a new middle (head) dim of size n
        return bass.AP(tensor=apv.tensor, offset=apv.offset,
                       ap=[list(apv.ap[0]), [0, n], [1, 64]])

    with TileContext(nc) as tc, ExitStack() as ctx:
        persist = ctx.enter_context(tc.tile_pool(name="persist", bufs=1))
        ones = persist.tile([128, 128], DT)
        ident = persist.tile([128, 128], DT)
        maskt_sb = persist.tile([128, 128], F32)
        bq_sb = persist.tile([1, 768], DT)
        # [d, seg, t]: segs 0..3 = q heads, seg 4 = k
        qkT = persist.tile([128, 5, T], DT)
        vbuf = persist.tile([128, TB, 128], DT)   # [t-in-block, j, d]
        attT = persist.tile([128, HQ, T], DT)     # [d, head, t]
        eps_sb = persist.tile([128, 1], F32)
        nc.gpsimd.memset(eps_sb, EPS)
        # touch Exp early so its ACT table is resident before P2
        warm = persist.tile([128, 1], F32)
        nc.scalar.activation(out=warm, in_=eps_sb, func=AF.Exp)
        # SBUF pools live for the whole kernel (no inter-phase release
        # barriers); only PSUM pools are phase-scoped since each phase
        # needs most of the 8 banks.
        p1 = ctx.enter_context(tc.tile_pool(name="p1", bufs=3))
        p1x = ctx.enter_context(tc.tile_pool(name="p1x", bufs=2))
        p1s = ctx.enter_context(tc.tile_pool(name="p1s", bufs=3))
        p1w = ctx.enter_context(tc.tile_pool(name="p1w", bufs=1))
        p2 = ctx.enter_context(tc.tile_pool(name="p2", bufs=3))
        p2d = ctx.enter_context(tc.tile_pool(name="p2d", bufs=8))
        p3 = ctx.enter_context(tc.tile_pool(name="p3", bufs=2))
        p3w = ctx.enter_context(tc.tile_pool(name="p3w", bufs=1))
        # ---------------- P1: QKV + RMSNorm + RoPE + transpose ----------
        with tc.tile_pool(name="p1ps", bufs=3, space="PSUM") as p1ps, \
             tc.tile_pool(name="scps", bufs=2, space="PSUM") as scps:
            # x/trig tiles stream on the SP queue; weights + constants go
            # on the ACT hwdge queue so the first QKV matmul only waits on
            # xtall(0) + wqkv chunk 0.
            xts = []
            trigs = []

            def fetch_xt(tt, chunked=False):
                xtall = p1x.tile([128, CB, 128], DT, tag="xt")
                if chunked:
                    # per-cc chunks so the first matmul waits only on cc=0
                    for cc in range(CB):
                        nc.sync.dma_start(out=xtall[:, cc, :],
                                          in_=xt[tt, :, cc * 128:(cc + 1) * 128])
                else:
                    nc.sync.dma_start(out=xtall, in_=xt[tt])
                trig_sb = p1x.tile([128, 8, 64], DT, tag="trig")
                nc.sync.dma_start(out=trig_sb, in_=trig[tt])
                xts.append(xtall)
                trigs.append(trig_sb)

            # startup: even weight chunks stream on the ACT queue while
            # the SP queue interleaves x chunks with odd weight chunks, so
            # QKV(0) consumes both streams at matmul pace
            wqkv_sb = p1w.tile([128, CB, 768], DT)
            xtall0 = p1x.tile([128, CB, 128], DT, tag="xt")
            trig_sb0 = p1x.tile([128, 8, 64], DT, tag="trig")
            xts.append(xtall0)
            trigs.append(trig_sb0)
            for cc in range(CB):
                if cc % 2 == 0:
                    nc.scalar.dma_start(out=wqkv_sb[:, cc, :], in_=wqkv[cc])
                nc.sync.dma_start(out=xtall0[:, cc, :],
                                  in_=xt[0, :, cc * 128:(cc + 1) * 128])
                if cc % 2 == 1:
                    nc.sync.dma_start(out=wqkv_sb[:, cc, :], in_=wqkv[cc])
            nc.sync.dma_start(out=trig_sb0, in_=trig[0])
            nc.scalar.dma_start(out=ones, in_=cst[0])
            nc.scalar.dma_start(out=ident, in_=cst[1])
            nc.scalar.dma_start(out=maskt_sb, in_=maskt[:, :])
            if use_bias:
                nc.scalar.dma_start(out=bq_sb, in_=bqkv[:, :])
            wo_sb = p3w.tile([128, HQ, C], DT)
            for h in range(HQ):
                nc.scalar.dma_start(out=wo_sb[:, h, :], in_=wo[h])

            qrs = [None] * TB

            def transpose_out(tt):
                qr = qrs[tt]
                # all 5 transposed segs share one bf16 PSUM bank, drained
                # by a single wide DVE copy into the strided qkT view
                tp = scps.tile([128, 5, 128], DT, tag="tp")
                for s in range(5):
                    nc.tensor.transpose(tp[:, s, :],
                                        qr[:, s * 128:(s + 1) * 128], ident)
                nc.vector.tensor_copy(qkT[:, :, tt * 128:(tt + 1) * 128], tp)

            for tt in range(TB):
                if tt + 1 < TB:
                    fetch_xt(tt + 1)
                xtall = xts[tt]
                trig_sb = trigs[tt]

                qkv_ps = p1ps.tile([128, 768], F32, tag="qkv")
                last = not use_bias
                for cc in range(CB):
                    nc.tensor.matmul(qkv_ps[:, 0:512], xtall[:, cc, :],
                                     wqkv_sb[:, cc, 0:512],
                                     start=(cc == 0),
                                     stop=(last and cc == CB - 1))
                    nc.tensor.matmul(qkv_ps[:, 512:768], xtall[:, cc, :],
                                     wqkv_sb[:, cc, 512:768],
                                     start=(cc == 0),
                                     stop=(last and cc == CB - 1))
                if use_bias:
                    nc.tensor.matmul(qkv_ps[:, 0:512], ones[0:1, :],
                                     bq_sb[0:1, 0:512], start=False, stop=True)
                    nc.tensor.matmul(qkv_ps[:, 512:768], ones[0:1, :],
                                     bq_sb[0:1, 512:768], start=False, stop=True)

                # software pipeline: PE transposes run two tiles behind so
                # the in-order PE never waits on the norm+rope chain.
                if tt >= 2:
                    transpose_out(tt - 2)

                # RMSNorm stats for 4 q heads + k
                ssq = p1s.tile([128, 8], F32, tag="ssq")
                for s in range(5):
                    sqs = p1s.tile([128, 128], F32, tag="sqs")
                    nc.scalar.activation(out=sqs, in_=qkv_ps[:, s * 128:(s + 1) * 128],
                                         func=AF.Square, accum_out=ssq[:, s:s + 1])
                # sqrt(mean_sq + eps) with the 1/D fold; rstd via the fast
                # 18-bit DVE reciprocal (plenty under bf16 matmul noise)
                sq5 = p1s.tile([128, 8], F32, tag="sq5")
                nc.scalar.activation(out=sq5[:, 0:5], in_=ssq[:, 0:5],
                                     func=AF.Sqrt, scale=1.0 / D, bias=eps_sb)
                rstd = p1s.tile([128, 8], F32, tag="rstd")
                nc.vector.reciprocal_approx_fast(out=rstd[:, 0:5], in_=sq5[:, 0:5])

                # normalize q/k (x * rstd); v copy rides the Pool engine
                qn = p1.tile([128, 640], F32, tag="qn")
                for s in range(5):
                    nc.scalar.activation(out=qn[:, s * 128:(s + 1) * 128],
                                         in_=qkv_ps[:, s * 128:(s + 1) * 128],
                                         func=AF.Copy, scale=rstd[:, s:s + 1], bias=0.0)
                nc.vector.tensor_copy(vbuf[:, tt, :], qkv_ps[:, 640:768])

                # RoPE (norm weights folded into trig tables host-side)
                qr = p1.tile([128, 640], DT, tag="qr")
                qrs[tt] = qr

                def rope(seg0, nseg, toff):
                    src = qn[:, seg0 * 128:(seg0 + nseg) * 128]
                    dst = qr[:, seg0 * 128:(seg0 + nseg) * 128]
                    sev = src.rearrange("p (h j t) -> p h t j", t=2, j=64)
                    dev = dst.rearrange("p (h j t) -> p h t j", t=2, j=64)
                    qe, qo = sev[:, :, 0, :], sev[:, :, 1, :]
                    re, ro = dev[:, :, 0, :], dev[:, :, 1, :]
                    ce = bc4(trig_sb[:, toff + 0, :], nseg)
                    so = bc4(trig_sb[:, toff + 1, :], nseg)
                    se = bc4(trig_sb[:, toff + 2, :], nseg)
                    co = bc4(trig_sb[:, toff + 3, :], nseg)
                    ta = p1s.tile([128, nseg, 64], F32, tag=f"ra{toff}")
                    tb = p1s.tile([128, nseg, 64], F32, tag=f"rb{toff}")
                    nc.vector.tensor_mul(ta, qe, ce)
                    nc.gpsimd.tensor_mul(tb, qo, so)
                    nc.vector.tensor_sub(re, ta, tb)
                    tc_ = p1s.tile([128, nseg, 64], F32, tag=f"rc{toff}")
                    td = p1s.tile([128, nseg, 64], F32, tag=f"rd{toff}")
                    nc.gpsimd.tensor_mul(tc_, qe, se)
                    nc.vector.tensor_mul(td, qo, co)
                    nc.gpsimd.tensor_add(ro, tc_, td)

                rope(0, 4, 0)   # q heads, tables 0..3
                rope(4, 1, 4)   # k, tables 4..7

            def emit_scores(h, q0, j, scpool, sctag, ptpool):
                qlo = max(q0, j * 128)
                w = q0 + 512 - qlo
                sc = scpool.tile(*(([128, 768], F32) if sctag == "qkv"
                                   else ([128, 512], F32)), tag=sctag)
                nc.tensor.matmul(sc[:, 0:w], qkT[:, 4, j * 128:(j + 1) * 128],
                                 qkT[:, h, qlo:q0 + 512],
                                 start=True, stop=True)
                if qlo == j * 128:
                    nc.vector.tensor_add(sc[:, 0:128], sc[:, 0:128], maskt_sb)
                pT = ptpool.tile([128, 512], DT, tag="pT")
                nc.scalar.activation(out=pT[:, 0:w], in_=sc[:, 0:w],
                                     func=AF.Exp, scale=SCALE)
                return pT, qlo, w

            # seam pipeline: the first two attention rounds' scores+exp
            # (borrowing the qkv PSUM ring for score tiles) keep the PE
            # busy while the last two tiles' rope chains drain; then the
            # PE transposes for those tiles run with no stall, and the
            # deferred PV/sums follow once p2acc opens.
            deferred = {h: [emit_scores(h, 0, j, p1ps, "qkv", p2d)
                            for j in range(4)]
                        for h in (0, 1)}
            transpose_out(TB - 2)
            transpose_out(TB - 1)

        # ---------------- P2: attention (transposed scores) -------------
        with tc.tile_pool(name="p2acc", bufs=2, space="PSUM") as p2acc, \
             tc.tile_pool(name="p2sc", bufs=3, space="PSUM") as p2sc:
            def pv_sums(outT, sums, q0, jmax, j, pT, qlo, w):
                o0 = qlo - q0
                nc.tensor.matmul(outT[:, o0:512],
                                 vbuf[:, j, :], pT[:, 0:w],
                                 start=(j == 0), stop=(j == jmax),
                                 skip_group_check=True)
                nc.tensor.matmul(sums[:, o0:512],
                                 ones, pT[:, 0:w],
                                 start=(j == 0), stop=(j == jmax),
                                 skip_group_check=True)

            def finish(h, q0, outT, sums):
                inv = p2.tile([128, 512], F32, tag="inv")
                nc.vector.reciprocal_approx_fast(out=inv, in_=sums)
                nc.vector.tensor_mul(attT[:, h, q0:q0 + 512], outT, inv)

            for h in (0, 1):
                outT = p2acc.tile([128, 512], F32, tag="outT")
                sums = p2acc.tile([128, 512], F32, tag="sums")
                for j in range(4):
                    pT, qlo, w = deferred[h][j]
                    pv_sums(outT, sums, 0, 3, j, pT, qlo, w)
                finish(h, 0, outT, sums)
            for h in range(HQ):
                for qc in range(QC):
                    if qc == 0 and h < 2:
                        continue
                    q0 = qc * 512
                    jmax = 4 * qc + 3
                    outT = p2acc.tile([128, 512], F32, tag="outT")
                    sums = p2acc.tile([128, 512], F32, tag="sums")
                    for j in range(jmax + 1):
                        pT, qlo, w = emit_scores(h, q0, j, p2sc, "sc", p2)
                        pv_sums(outT, sums, q0, jmax, j, pT, qlo, w)
                    finish(h, q0, outT, sums)

        # ---------------- P3: output projection --------------------------
        with tc.tile_pool(name="p3ps", bufs=2, space="PSUM") as p3ps:
            for tt in range(TB):
                y_ps = p3ps.tile([128, 2048], F32, tag="y")
                for h in range(HQ):
                    for c4 in range(4):
                        nc.tensor.matmul(y_ps[:, c4 * 512:(c4 + 1) * 512],
                                         attT[:, h, tt * 128:(tt + 1) * 128],
                                         wo_sb[:, h, c4 * 512:(c4 + 1) * 512],
                                         start=(h == 0), stop=(h == HQ - 1))
                y_sb = p3.tile([128, 2048], DT, tag="ysb")
                nc.scalar.activation(out=y_sb[:, 0:512], in_=y_ps[:, 0:512],
                                     func=AF.Copy, scale=1.0, bias=0.0)
                nc.vector.tensor_copy(y_sb[:, 512:2048], y_ps[:, 512:2048])
                nc.sync.dma_start(out=y[tt * 128:(tt + 1) * 128, :], in_=y_sb)

    nc.compile()
    return nc


def _np_dt(dt_name):
    if dt_name == "bfloat16":
        import ml_dtypes
        return np.dtype(ml_dtypes.bfloat16)
    return np.dtype(np.float32)


def _prep_core_inputs(b, g, x, Wq, bq, Wk, bk, Wv, bv, Wo, bo, qn_w, kn_w,
                      freqs_cos, freqs_sin, mask, dt=np.float32):
    f32 = np.float32
    xb = np.ascontiguousarray(x[b], dtype=f32)
    # [tt, csub, cc, tcol]: xt[tt][p][cc*128+tc] = x[b][tt*128+tc][cc*128+p]
    xt = np.ascontiguousarray(
        xb.reshape(TB, 128, CB, 128).transpose(0, 3, 2, 1)
    ).reshape(TB, 128, CB * 128)
    wqkv = np.ascontiguousarray(np.concatenate([
        Wq[:, g * 512:(g + 1) * 512],
        Wk[:, g * 128:(g + 1) * 128],
        Wv[:, g * 128:(g + 1) * 128],
    ], axis=1).reshape(CB, 128, 768), dtype=f32)
    bqkv = np.concatenate([
        bq[g * 512:(g + 1) * 512], bk[g * 128:(g + 1) * 128],
        bv[g * 128:(g + 1) * 128],
    ]).reshape(1, 768).astype(f32)
    cos = freqs_cos.astype(f32)
    sin = freqs_sin.astype(f32)
    qe, qo = qn_w[0::2].astype(f32), qn_w[1::2].astype(f32)
    ke, ko = kn_w[0::2].astype(f32), kn_w[1::2].astype(f32)
    # tables: [ce, so, se, co] for q then for k; layout [TB, 128, 8*64]
    tabs = np.stack([cos * qe, sin * qo, sin * qe, cos * qo,
                     cos * ke, sin * ko, sin * ke, cos * ko], axis=1)  # [T, 8, 64]
    trig = np.ascontiguousarray(tabs.reshape(TB, 128, 8 * 64), dtype=f32)
    maskt = np.ascontiguousarray(mask[0, 0, :128, :128].T, dtype=f32)
    cst = np.stack([np.ones((128, 128), f32), np.eye(128, dtype=f32)])
    wo_t = np.ascontiguousarray(
        Wo[g * 512:(g + 1) * 512].reshape(HQ, 128, C), dtype=f32)
    out = {"xt": xt, "wqkv": wqkv, "bqkv": bqkv, "trig": trig,
           "maskt": maskt, "cst": cst, "wo": wo_t}
    if np.dtype(dt) != np.float32:
        out = {k: (v if k == "maskt" else np.ascontiguousarray(v.astype(dt)))
               for k, v in out.items()}
    return out


def kernel(x, Wq, bq, Wk, bk, Wv, bv, Wo, bo, qn_w, kn_w,
           freqs_cos, freqs_sin, mask, _trace=False, _trace_kwargs=None):
    from concourse.bass_utils import run_bass_kernel_spmd

    args = (np.asarray(x), np.asarray(Wq), np.asarray(bq), np.asarray(Wk),
            np.asarray(bk), np.asarray(Wv), np.asarray(bv), np.asarray(Wo),
            np.asarray(bo), np.asarray(qn_w), np.asarray(kn_w),
            np.asarray(freqs_cos), np.asarray(freqs_sin), np.asarray(mask))
    bo_np = args[8].astype(np.float32)
    use_bias = bool(np.any(args[2]) or np.any(args[4]) or np.any(args[6]))

    dt_name = os.environ.get("BASS_ATTN_DT", "bfloat16")
    key = (dt_name, use_bias)
    if key not in _CACHE:
        _CACHE[key] = _build(dt_name, use_bias)
    nc = _CACHE[key]

    np_dt = _np_dt(dt_name)
    in_maps = [_prep_core_inputs(cid // 4, cid % 4, *args, dt=np_dt)
               for cid in range(8)]
    res = run_bass_kernel_spmd(nc, in_maps, core_ids=list(range(8)),
                               trace=_trace, **(_trace_kwargs or {}))
    outs = [np.asarray(res.results[i]["y"], dtype=np.float32) for i in range(8)]
    yfull = np.empty((B, T, C), dtype=np.float32)
    for b in range(B):
        yfull[b] = outs[4 * b] + outs[4 * b + 1] + outs[4 * b + 2] + outs[4 * b + 3]
        yfull[b] += bo_np[None, :]
    if _trace:
        kernel._last_result = res
    return yfull


# revision 59
# speedup vs baseline: 1.2076x; 1.2076x over previous
"""LLaMA causal self-attention (GQA) on 8 Trainium2 NeuronCores.

Sharding: 2-way data-parallel over batch x 4-way tensor-parallel over KV
groups. Core cid handles batch b=cid//4 and KV group g=cid%4 (q heads
4g..4g+3, kv head g). Each core computes a partial output y_partial =
att_heads @ Wo_rows; the host sums the 4 partials per batch and adds bo.

Per-core pipeline (all layouts chosen so matmul contraction is on the
partition dim and softmax needs no transposes):
  P1: QKV projection (x^T chunks stationary), optional bias via K=1
      ones-row matmul, RMSNorm (ACT Square+accum, Newton-refined rsqrt),
      RoPE (elementwise, with qn_w/kn_w folded into the trig tables).
      The PE-transposes of q/k to [d, t] layout for tile tt are issued
      after tile tt+1's QKV matmuls so the in-order PE never stalls on
      the ACT/DVE norm+rope chain; PSUM->SBUF copies ride the Pool
      engine.
  P2: attention computed transposed: scoresT[k, q] = kT_j^T @ qT in
      512-column q chunks, additive causal mask on diagonal blocks, exp
      without max subtraction (RMS-normed scores are bounded, softmax is
      shift invariant), softmax denominators via an all-ones stationary
      matmul (sums land broadcast across all partitions), PV accumulated
      in PSUM over j. Accumulator PSUM is double-buffered so the DVE
      normalize (reciprocal+mul) of chunk c overlaps the matmuls of
      chunk c+1.
  P3: output projection from attT chunks, PSUM -> SBUF -> DRAM.

Matmul operands are bf16 (PE runs 2x the fp32r column rate); softmax
statistics and PSUM accumulation stay fp32.
"""

import os
from contextlib import ExitStack

import numpy as np

B, T, C = 2, 2048, 2048
H, KV = 16, 4
D = 128
HQ = H // KV        # q heads per core = 4
TB = T // 128       # 16
CB = C // 128       # 16
EPS = 1e-5
SCALE = float(np.float32(1.0) / np.sqrt(np.float32(D)))
QC = 4              # q chunks of 512 per head in P2

_CACHE = {}


def _build(dt_name, use_bias):
    import concourse.bass as bass
    import concourse.bacc as bacc
    from concourse import mybir
    from concourse.tile import TileContext

    DT = getattr(mybir.dt, dt_name)
    F32 = mybir.dt.float32
    AF = mybir.ActivationFunctionType
    ALU = mybir.AluOpType

    nc = bacc.Bacc(None, target_bir_lowering=False)
    xt = nc.dram_tensor("xt", [TB, 128, CB * 128], DT, kind="ExternalInput")
    wqkv = nc.dram_tensor("wqkv", [CB, 128, 768], DT, kind="ExternalInput")
    bqkv = nc.dram_tensor("bqkv", [1, 768], DT, kind="ExternalInput")
    trig = nc.dram_tensor("trig", [TB, 128, 8 * 64], DT, kind="ExternalInput")
    maskt = nc.dram_tensor("maskt", [128, 128], F32, kind="ExternalInput")
    cst = nc.dram_tensor("cst", [2, 128, 128], DT, kind="ExternalInput")
    wo = nc.dram_tensor("wo", [HQ, 128, C], DT, kind="ExternalInput")
    y = nc.dram_tensor("y", [T, C], DT, kind="ExternalOutput")

    def bc4(apv, n):
        # broadcast a [128, 64] AP along a new middle (head) dim of size n
        return bass.AP(tensor=apv.tensor, offset=apv.offset,
                       ap=[list(apv.ap[0]), [0, n], [1, 64]])

    with TileContext(nc) as tc, ExitStack() as ctx:
        persist = ctx.enter_context(tc.tile_pool(name="persist", bufs=1))
        ones = persist.tile([128, 128], DT)
        ident = persist.tile([128, 128], DT)
        maskt_sb = persist.tile([128, 128], F32)
        bq_sb = persist.tile([1, 768], DT)
        # [d, seg, t]: segs 0..3 = q heads, seg 4 = k
        qkT = persist.tile([128, 5, T], DT)
        vbuf = persist.tile([128, TB, 128], DT)   # [t-in-block, j, d]
        attT = persist.tile([128, HQ, T], DT)     # [d, head, t]
        eps_sb = persist.tile([128, 1], F32)
        nc.gpsimd.memset(eps_sb, EPS)
        # touch Exp early so its ACT table is resident before P2
        warm = persist.tile([128, 1], F32)
        nc.scalar.activation(out=warm, in_=eps_sb, func=AF.Exp)
        # SBUF pools live for the whole kernel (no inter-phase release
        # barriers); only PSUM pools are phase-scoped since each phase
        # needs most of the 8 banks.
        p1 = ctx.enter_context(tc.tile_pool(name="p1", bufs=3))
        p1x = ctx.enter_context(tc.tile_pool(name="p1x", bufs=3))
        p1s = ctx.enter_context(tc.tile_pool(name="p1s", bufs=3))
        p1w = ctx.enter_context(tc.tile_pool(name="p1w", bufs=1))
        p2 = ctx.enter_context(tc.tile_pool(name="p2", bufs=3))
        p2d = ctx.enter_context(tc.tile_pool(name="p2d", bufs=8))
        p3 = ctx.enter_context(tc.tile_pool(name="p3", bufs=2))
        p3w = ctx.enter_context(tc.tile_pool(name="p3w", bufs=1))
        # ---------------- P1: QKV + RMSNorm + RoPE + transpose ----------
        with tc.tile_pool(name="p1ps", bufs=3, space="PSUM") as p1ps, \
             tc.tile_pool(name="scps", bufs=2, space="PSUM") as scps:
            # x/trig tiles stream on the SP queue; weights + constants go
            # on the ACT hwdge queue so the first QKV matmul only waits on
            # xtall(0) + wqkv chunk 0.
            xts = []
            trigs = []

            def fetch_xt(tt, chunked=False):
                xtall = p1x.tile([128, CB, 128], DT, tag="xt")
                if chunked:
                    # per-cc chunks so the first matmul waits only on cc=0
                    for cc in range(CB):
                        nc.sync.dma_start(out=xtall[:, cc, :],
                                          in_=xt[tt, :, cc * 128:(cc + 1) * 128])
                else:
                    nc.sync.dma_start(out=xtall, in_=xt[tt])
                trig_sb = p1x.tile([128, 8, 64], DT, tag="trig")
                nc.sync.dma_start(out=trig_sb, in_=trig[tt])
                xts.append(xtall)
                trigs.append(trig_sb)

            fetch_xt(0, chunked=True)
            wqkv_sb = p1w.tile([128, CB, 768], DT)
            for cc in range(CB):
                nc.scalar.dma_start(out=wqkv_sb[:, cc, :], in_=wqkv[cc])
            nc.scalar.dma_start(out=ones, in_=cst[0])
            nc.scalar.dma_start(out=ident, in_=cst[1])
            nc.scalar.dma_start(out=maskt_sb, in_=maskt[:, :])
            if use_bias:
                nc.scalar.dma_start(out=bq_sb, in_=bqkv[:, :])
            wo_sb = p3w.tile([128, HQ, C], DT)
            for h in range(HQ):
                nc.scalar.dma_start(out=wo_sb[:, h, :], in_=wo[h])

            qrs = [None] * TB

            def transpose_out(tt):
                qr = qrs[tt]
                # all 5 transposed segs share one bf16 PSUM bank, drained
                # by a single wide DVE copy into the strided qkT view
                tp = scps.tile([128, 5, 128], DT, tag="tp")
                for s in range(5):
                    nc.tensor.transpose(tp[:, s, :],
                                        qr[:, s * 128:(s + 1) * 128], ident)
                nc.vector.tensor_copy(qkT[:, :, tt * 128:(tt + 1) * 128], tp)

            for tt in range(TB):
                if tt + 1 < TB:
                    fetch_xt(tt + 1)
                xtall = xts[tt]
                trig_sb = trigs[tt]

                qkv_ps = p1ps.tile([128, 768], F32, tag="qkv")
                last = not use_bias
                for cc in range(CB):
                    nc.tensor.matmul(qkv_ps[:, 0:512], xtall[:, cc, :],
                                     wqkv_sb[:, cc, 0:512],
                                     start=(cc == 0),
                                     stop=(last and cc == CB - 1))
                    nc.tensor.matmul(qkv_ps[:, 512:768], xtall[:, cc, :],
                                     wqkv_sb[:, cc, 512:768],
                                     start=(cc == 0),
                                     stop=(last and cc == CB - 1))
                if use_bias:
                    nc.tensor.matmul(qkv_ps[:, 0:512], ones[0:1, :],
                                     bq_sb[0:1, 0:512], start=False, stop=True)
                    nc.tensor.matmul(qkv_ps[:, 512:768], ones[0:1, :],
                                     bq_sb[0:1, 512:768], start=False, stop=True)

                # software pipeline: PE transposes run two tiles behind so
                # the in-order PE never waits on the norm+rope chain.
                if tt >= 2:
                    transpose_out(tt - 2)

                # RMSNorm stats for 4 q heads + k
                ssq = p1s.tile([128, 8], F32, tag="ssq")
                for s in range(5):
                    sqs = p1s.tile([128, 128], F32, tag="sqs")
                    nc.scalar.activation(out=sqs, in_=qkv_ps[:, s * 128:(s + 1) * 128],
                                         func=AF.Square, accum_out=ssq[:, s:s + 1])
                # sqrt(mean_sq + eps) with the 1/D fold; rstd via the fast
                # 18-bit DVE reciprocal (plenty under bf16 matmul noise)
                sq5 = p1s.tile([128, 8], F32, tag="sq5")
                nc.scalar.activation(out=sq5[:, 0:5], in_=ssq[:, 0:5],
                                     func=AF.Sqrt, scale=1.0 / D, bias=eps_sb)
                rstd = p1s.tile([128, 8], F32, tag="rstd")
                nc.vector.reciprocal_approx_fast(out=rstd[:, 0:5], in_=sq5[:, 0:5])

                # normalize q/k (x * rstd); v copy rides the Pool engine
                qn = p1.tile([128, 640], F32, tag="qn")
                for s in range(5):
                    nc.scalar.activation(out=qn[:, s * 128:(s + 1) * 128],
                                         in_=qkv_ps[:, s * 128:(s + 1) * 128],
                                         func=AF.Copy, scale=rstd[:, s:s + 1], bias=0.0)
                nc.vector.tensor_copy(vbuf[:, tt, :], qkv_ps[:, 640:768])

                # RoPE (norm weights folded into trig tables host-side)
                qr = p1.tile([128, 640], DT, tag="qr")
                qrs[tt] = qr

                def rope(seg0, nseg, toff):
                    src = qn[:, seg0 * 128:(seg0 + nseg) * 128]
                    dst = qr[:, seg0 * 128:(seg0 + nseg) * 128]
                    sev = src.rearrange("p (h j t) -> p h t j", t=2, j=64)
                    dev = dst.rearrange("p (h j t) -> p h t j", t=2, j=64)
                    qe, qo = sev[:, :, 0, :], sev[:, :, 1, :]
                    re, ro = dev[:, :, 0, :], dev[:, :, 1, :]
                    ce = bc4(trig_sb[:, toff + 0, :], nseg)
                    so = bc4(trig_sb[:, toff + 1, :], nseg)
                    se = bc4(trig_sb[:, toff + 2, :], nseg)
                    co = bc4(trig_sb[:, toff + 3, :], nseg)
                    ta = p1s.tile([128, nseg, 64], F32, tag=f"ra{toff}")
                    tb = p1s.tile([128, nseg, 64], F32, tag=f"rb{toff}")
                    nc.vector.tensor_mul(ta, qe, ce)
                    nc.gpsimd.tensor_mul(tb, qo, so)
                    nc.vector.tensor_sub(re, ta, tb)
                    tc_ = p1s.tile([128, nseg, 64], F32, tag=f"rc{toff}")
                    td = p1s.tile([128, nseg, 64], F32, tag=f"rd{toff}")
                    nc.gpsimd.tensor_mul(tc_, qe, se)
                    nc.vector.tensor_mul(td, qo, co)
                    nc.gpsimd.tensor_add(ro, tc_, td)

                rope(0, 4, 0)   # q heads, tables 0..3
                rope(4, 1, 4)   # k, tables 4..7

            def emit_scores(h, q0, j, scpool, sctag, ptpool):
                qlo = max(q0, j * 128)
                w = q0 + 512 - qlo
                sc = scpool.tile(*(([128, 768], F32) if sctag == "qkv"
                                   else ([128, 512], F32)), tag=sctag)
                nc.tensor.matmul(sc[:, 0:w], qkT[:, 4, j * 128:(j + 1) * 128],
                                 qkT[:, h, qlo:q0 + 512],
                                 start=True, stop=True)
                if qlo == j * 128:
                    nc.vector.tensor_add(sc[:, 0:128], sc[:, 0:128], maskt_sb)
                pT = ptpool.tile([128, 512], DT, tag="pT")
                nc.scalar.activation(out=pT[:, 0:w], in_=sc[:, 0:w],
                                     func=AF.Exp, scale=SCALE)
                return pT, qlo, w

            # seam pipeline: the first two attention rounds' scores+exp
            # (borrowing the qkv PSUM ring for score tiles) keep the PE
            # busy while the last two tiles' rope chains drain; then the
            # PE transposes for those tiles run with no stall, and the
            # deferred PV/sums follow once p2acc opens.
            deferred = {h: [emit_scores(h, 0, j, p1ps, "qkv", p2d)
                            for j in range(4)]
                        for h in (0, 1)}
            transpose_out(TB - 2)
            transpose_out(TB - 1)

        # ---------------- P2: attention (transposed scores) -------------
        with tc.tile_pool(name="p2acc", bufs=2, space="PSUM") as p2acc, \
             tc.tile_pool(name="p2sc", bufs=4, space="PSUM") as p2sc:
            def pv_sums(outT, sums, q0, jmax, j, pT, qlo, w):
                o0 = qlo - q0
                nc.tensor.matmul(outT[:, o0:512],
                                 vbuf[:, j, :], pT[:, 0:w],
                                 start=(j == 0), stop=(j == jmax),
                                 skip_group_check=True)
                nc.tensor.matmul(sums[:, o0:512],
                                 ones, pT[:, 0:w],
                                 start=(j == 0), stop=(j == jmax),
                                 skip_group_check=True)

            def finish(h, q0, outT, sums):
                inv = p2.tile([128, 512], F32, tag="inv")
                nc.vector.reciprocal_approx_fast(out=inv, in_=sums)
                nc.vector.tensor_mul(attT[:, h, q0:q0 + 512], outT, inv)

            for h in (0, 1):
                outT = p2acc.tile([128, 512], F32, tag="outT")
                sums = p2acc.tile([128, 512], F32, tag="sums")
                for j in range(4):
                    pT, qlo, w = deferred[h][j]
                    pv_sums(outT, sums, 0, 3, j, pT, qlo, w)
                finish(h, 0, outT, sums)
            for h in range(HQ):
                for qc in range(QC):
                    if qc == 0 and h < 2:
                        continue
                    q0 = qc * 512
                    jmax = 4 * qc + 3
                    outT = p2acc.tile([128, 512], F32, tag="outT")
                    sums = p2acc.tile([128, 512], F32, tag="sums")
                    for j in range(jmax + 1):
                        pT, qlo, w = emit_scores(h, q0, j, p2sc, "sc", p2)
                        pv_sums(outT, sums, q0, jmax, j, pT, qlo, w)
                    finish(h, q0, outT, sums)

        # ---------------- P3: output projection --------------------------
        with tc.tile_pool(name="p3ps", bufs=2, space="PSUM") as p3ps:
            for tt in range(TB):
                y_ps = p3ps.tile([128, 2048], F32, tag="y")
                for h in range(HQ):
                    for c4 in range(4):
                        nc.tensor.matmul(y_ps[:, c4 * 512:(c4 + 1) * 512],
                                         attT[:, h, tt * 128:(tt + 1) * 128],
                                         wo_sb[:, h, c4 * 512:(c4 + 1) * 512],
                                         start=(h == 0), stop=(h == HQ - 1))
                y_sb = p3.tile([128, 2048], DT, tag="ysb")
                nc.scalar.activation(out=y_sb[:, 0:512], in_=y_ps[:, 0:512],
                                     func=AF.Copy, scale=1.0, bias=0.0)
                nc.vector.tensor_copy(y_sb[:, 512:2048], y_ps[:, 512:2048])
                nc.sync.dma_start(out=y[tt * 128:(tt + 1) * 128, :], in_=y_sb)

    nc.compile()
    return nc


def _np_dt(dt_name):
    if dt_name == "bfloat16":
        import ml_dtypes
        return np.dtype(ml_dtypes.bfloat16)
    return np.dtype(np.float32)


def _prep_core_inputs(b, g, x, Wq, bq, Wk, bk, Wv, bv, Wo, bo, qn_w, kn_w,
                      freqs_cos, freqs_sin, mask, dt=np.float32):
    f32 = np.float32
    xb = np.ascontiguousarray(x[b], dtype=f32)
    # [tt, csub, cc, tcol]: xt[tt][p][cc*128+tc] = x[b][tt*128+tc][cc*128+p]
    xt = np.ascontiguousarray(
        xb.reshape(TB, 128, CB, 128).transpose(0, 3, 2, 1)
    ).reshape(TB, 128, CB * 128)
    wqkv = np.ascontiguousarray(np.concatenate([
        Wq[:, g * 512:(g + 1) * 512],
        Wk[:, g * 128:(g + 1) * 128],
        Wv[:, g * 128:(g + 1) * 128],
    ], axis=1).reshape(CB, 128, 768), dtype=f32)
    bqkv = np.concatenate([
        bq[g * 512:(g + 1) * 512], bk[g * 128:(g + 1) * 128],
        bv[g * 128:(g + 1) * 128],
    ]).reshape(1, 768).astype(f32)
    cos = freqs_cos.astype(f32)
    sin = freqs_sin.astype(f32)
    qe, qo = qn_w[0::2].astype(f32), qn_w[1::2].astype(f32)
    ke, ko = kn_w[0::2].astype(f32), kn_w[1::2].astype(f32)
    # tables: [ce, so, se, co] for q then for k; layout [TB, 128, 8*64]
    tabs = np.stack([cos * qe, sin * qo, sin * qe, cos * qo,
                     cos * ke, sin * ko, sin * ke, cos * ko], axis=1)  # [T, 8, 64]
    trig = np.ascontiguousarray(tabs.reshape(TB, 128, 8 * 64), dtype=f32)
    maskt = np.ascontiguousarray(mask[0, 0, :128, :128].T, dtype=f32)
    cst = np.stack([np.ones((128, 128), f32), np.eye(128, dtype=f32)])
    wo_t = np.ascontiguousarray(
        Wo[g * 512:(g + 1) * 512].reshape(HQ, 128, C), dtype=f32)
    out = {"xt": xt, "wqkv": wqkv, "bqkv": bqkv, "trig": trig,
           "maskt": maskt, "cst": cst, "wo": wo_t}
    if np.dtype(dt) != np.float32:
        out = {k: (v if k == "maskt" else np.ascontiguousarray(v.astype(dt)))
               for k, v in out.items()}
    return out


def kernel(x, Wq, bq, Wk, bk, Wv, bv, Wo, bo, qn_w, kn_w,
           freqs_cos, freqs_sin, mask, _trace=False, _trace_kwargs=None):
    from concourse.bass_utils import run_bass_kernel_spmd

    args = (np.asarray(x), np.asarray(Wq), np.asarray(bq), np.asarray(Wk),
            np.asarray(bk), np.asarray(Wv), np.asarray(bv), np.asarray(Wo),
            np.asarray(bo), np.asarray(qn_w), np.asarray(kn_w),
            np.asarray(freqs_cos), np.asarray(freqs_sin), np.asarray(mask))
    bo_np = args[8].astype(np.float32)
    use_bias = bool(np.any(args[2]) or np.any(args[4]) or np.any(args[6]))

    dt_name = os.environ.get("BASS_ATTN_DT", "bfloat16")
    key = (dt_name, use_bias)
    if key not in _CACHE:
        _CACHE[key] = _build(dt_name, use_bias)
    nc = _CACHE[key]

    np_dt = _np_dt(dt_name)
    in_maps = [_prep_core_inputs(cid // 4, cid % 4, *args, dt=np_dt)
               for cid in range(8)]
    res = run_bass_kernel_spmd(nc, in_maps, core_ids=list(range(8)),
                               trace=_trace, **(_trace_kwargs or {}))
    outs = [np.asarray(res.results[i]["y"], dtype=np.float32) for i in range(8)]
    yfull = np.empty((B, T, C), dtype=np.float32)
    for b in range(B):
        yfull[b] = outs[4 * b] + outs[4 * b + 1] + outs[4 * b + 2] + outs[4 * b + 3]
        yfull[b] += bo_np[None, :]
    if _trace:
        kernel._last_result = res
    return yfull


# revision 60
# speedup vs baseline: 1.2260x; 1.0152x over previous
"""LLaMA causal self-attention (GQA) on 8 Trainium2 NeuronCores.

Sharding: 2-way data-parallel over batch x 4-way tensor-parallel over KV
groups. Core cid handles batch b=cid//4 and KV group g=cid%4 (q heads
4g..4g+3, kv head g). Each core computes a partial output y_partial =
att_heads @ Wo_rows; the host sums the 4 partials per batch and adds bo.

Per-core pipeline (all layouts chosen so matmul contraction is on the
partition dim and softmax needs no transposes):
  P1: QKV projection (x^T chunks stationary), optional bias via K=1
      ones-row matmul, RMSNorm (ACT Square+accum, Newton-refined rsqrt),
      RoPE (elementwise, with qn_w/kn_w folded into the trig tables).
      The PE-transposes of q/k to [d, t] layout for tile tt are issued
      after tile tt+1's QKV matmuls so the in-order PE never stalls on
      the ACT/DVE norm+rope chain; PSUM->SBUF copies ride the Pool
      engine.
  P2: attention computed transposed: scoresT[k, q] = kT_j^T @ qT in
      512-column q chunks, additive causal mask on diagonal blocks, exp
      without max subtraction (RMS-normed scores are bounded, softmax is
      shift invariant), softmax denominators via an all-ones stationary
      matmul (sums land broadcast across all partitions), PV accumulated
      in PSUM over j. Accumulator PSUM is double-buffered so the DVE
      normalize (reciprocal+mul) of chunk c overlaps the matmuls of
      chunk c+1.
  P3: output projection from attT chunks, PSUM -> SBUF -> DRAM.

Matmul operands are bf16 (PE runs 2x the fp32r column rate); softmax
statistics and PSUM accumulation stay fp32.
"""

import os
from contextlib import ExitStack

import numpy as np

B, T, C = 2, 2048, 2048
H, KV = 16, 4
D = 128
HQ = H // KV        # q heads per core = 4
TB = T // 128       # 16
CB = C // 128       # 16
EPS = 1e-5
SCALE = float(np.float32(1.0) / np.sqrt(np.float32(D)))
QC = 4              # q chunks of 512 per head in P2

_CACHE = {}


def _build(dt_name, use_bias):
    import concourse.bass as bass
    import concourse.bacc as bacc
    from concourse import mybir
    from concourse.tile import TileContext

    DT = getattr(mybir.dt, dt_name)
    F32 = mybir.dt.float32
    AF = mybir.ActivationFunctionType
    ALU = mybir.AluOpType

    nc = bacc.Bacc(None, target_bir_lowering=False)
    xt = nc.dram_tensor("xt", [TB, 128, CB * 128], DT, kind="ExternalInput")
    wqkv = nc.dram_tensor("wqkv", [CB, 128, 768], DT, kind="ExternalInput")
    bqkv = nc.dram_tensor("bqkv", [1, 768], DT, kind="ExternalInput")
    trig = nc.dram_tensor("trig", [TB, 128, 8 * 64], DT, kind="ExternalInput")
    maskt = nc.dram_tensor("maskt", [128, 128], F32, kind="ExternalInput")
    cst = nc.dram_tensor("cst", [2, 128, 128], DT, kind="ExternalInput")
    wo = nc.dram_tensor("wo", [HQ, 128, C], DT, kind="ExternalInput")
    y = nc.dram_tensor("y", [T, C], DT, kind="ExternalOutput")

    def bc4(apv, n):
        # broadcast a [128, 64] AP along a new middle (head) dim of size n
        return bass.AP(tensor=apv.tensor, offset=apv.offset,
                       ap=[list(apv.ap[0]), [0, n], [1, 64]])

    with TileContext(nc) as tc, ExitStack() as ctx:
        persist = ctx.enter_context(tc.tile_pool(name="persist", bufs=1))
        ones = persist.tile([128, 128], DT)
        ident = persist.tile([128, 128], DT)
        maskt_sb = persist.tile([128, 128], F32)
        bq_sb = persist.tile([1, 768], DT)
        # [d, seg, t]: segs 0..3 = q heads, seg 4 = k
        qkT = persist.tile([128, 5, T], DT)
        vbuf = persist.tile([128, TB, 128], DT)   # [t-in-block, j, d]
        attT = persist.tile([128, HQ, T], DT)     # [d, head, t]
        eps_sb = persist.tile([128, 1], F32)
        nc.gpsimd.memset(eps_sb, EPS)
        warm = persist.tile([128, 1], F32)
        # SBUF pools live for the whole kernel (no inter-phase release
        # barriers); only PSUM pools are phase-scoped since each phase
        # needs most of the 8 banks.
        p1 = ctx.enter_context(tc.tile_pool(name="p1", bufs=3))
        p1x = ctx.enter_context(tc.tile_pool(name="p1x", bufs=3))
        p1s = ctx.enter_context(tc.tile_pool(name="p1s", bufs=3))
        p1w = ctx.enter_context(tc.tile_pool(name="p1w", bufs=1))
        p2 = ctx.enter_context(tc.tile_pool(name="p2", bufs=3))
        p2d = ctx.enter_context(tc.tile_pool(name="p2d", bufs=8))
        p3 = ctx.enter_context(tc.tile_pool(name="p3", bufs=2))
        p3w = ctx.enter_context(tc.tile_pool(name="p3w", bufs=1))
        # ---------------- P1: QKV + RMSNorm + RoPE + transpose ----------
        with tc.tile_pool(name="p1ps", bufs=3, space="PSUM") as p1ps, \
             tc.tile_pool(name="scps", bufs=2, space="PSUM") as scps:
            # x/trig tiles stream on the SP queue; weights + constants go
            # on the ACT hwdge queue so the first QKV matmul only waits on
            # xtall(0) + wqkv chunk 0.
            xts = []
            trigs = []

            def fetch_xt(tt, chunked=False):
                xtall = p1x.tile([128, CB, 128], DT, tag="xt")
                if chunked:
                    # per-cc chunks so the first matmul waits only on cc=0
                    for cc in range(CB):
                        nc.sync.dma_start(out=xtall[:, cc, :],
                                          in_=xt[tt, :, cc * 128:(cc + 1) * 128])
                else:
                    nc.sync.dma_start(out=xtall, in_=xt[tt])
                trig_sb = p1x.tile([128, 8, 64], DT, tag="trig")
                nc.sync.dma_start(out=trig_sb, in_=trig[tt])
                xts.append(xtall)
                trigs.append(trig_sb)

            fetch_xt(0, chunked=True)
            wqkv_sb = p1w.tile([128, CB, 768], DT)
            for cc in range(0, CB, 2):
                nc.scalar.dma_start(
                    out=wqkv_sb[:, cc:cc + 2, :],
                    in_=wqkv[cc:cc + 2].rearrange("c p k -> p c k"))
            nc.scalar.dma_start(out=ones, in_=cst[0])
            nc.scalar.dma_start(out=ident, in_=cst[1])
            nc.scalar.dma_start(out=maskt_sb, in_=maskt[:, :])
            if use_bias:
                nc.scalar.dma_start(out=bq_sb, in_=bqkv[:, :])
            wo_sb = p3w.tile([128, HQ, C], DT)
            for h in range(HQ):
                nc.scalar.dma_start(out=wo_sb[:, h, :], in_=wo[h])

            qrs = [None] * TB

            def transpose_out(tt):
                qr = qrs[tt]
                # all 5 transposed segs share one bf16 PSUM bank, drained
                # by a single wide DVE copy into the strided qkT view
                tp = scps.tile([128, 5, 128], DT, tag="tp")
                for s in range(5):
                    nc.tensor.transpose(tp[:, s, :],
                                        qr[:, s * 128:(s + 1) * 128], ident)
                nc.vector.tensor_copy(qkT[:, :, tt * 128:(tt + 1) * 128], tp)

            for tt in range(TB):
                if tt + 1 < TB:
                    fetch_xt(tt + 1)
                xtall = xts[tt]
                trig_sb = trigs[tt]

                qkv_ps = p1ps.tile([128, 768], F32, tag="qkv")
                last = not use_bias
                for cc in range(CB):
                    nc.tensor.matmul(qkv_ps[:, 0:512], xtall[:, cc, :],
                                     wqkv_sb[:, cc, 0:512],
                                     start=(cc == 0),
                                     stop=(last and cc == CB - 1))
                    nc.tensor.matmul(qkv_ps[:, 512:768], xtall[:, cc, :],
                                     wqkv_sb[:, cc, 512:768],
                                     start=(cc == 0),
                                     stop=(last and cc == CB - 1))
                if use_bias:
                    nc.tensor.matmul(qkv_ps[:, 0:512], ones[0:1, :],
                                     bq_sb[0:1, 0:512], start=False, stop=True)
                    nc.tensor.matmul(qkv_ps[:, 512:768], ones[0:1, :],
                                     bq_sb[0:1, 512:768], start=False, stop=True)

                # software pipeline: PE transposes run two tiles behind so
                # the in-order PE never waits on the norm+rope chain.
                if tt >= 2:
                    transpose_out(tt - 2)

                # RMSNorm stats for 4 q heads + k
                ssq = p1s.tile([128, 8], F32, tag="ssq")
                for s in range(5):
                    sqs = p1s.tile([128, 128], F32, tag="sqs")
                    nc.scalar.activation(out=sqs, in_=qkv_ps[:, s * 128:(s + 1) * 128],
                                         func=AF.Square, accum_out=ssq[:, s:s + 1])
                # sqrt(mean_sq + eps) with the 1/D fold; rstd via the fast
                # 18-bit DVE reciprocal (plenty under bf16 matmul noise)
                sq5 = p1s.tile([128, 8], F32, tag="sq5")
                nc.scalar.activation(out=sq5[:, 0:5], in_=ssq[:, 0:5],
                                     func=AF.Sqrt, scale=1.0 / D, bias=eps_sb)
                rstd = p1s.tile([128, 8], F32, tag="rstd")
                nc.vector.reciprocal_approx_fast(out=rstd[:, 0:5], in_=sq5[:, 0:5])
                if tt == TB - 1:
                    # after the last Sqrt: pull the Exp table load off the
                    # P1->P2 seam (norm/rope below use the table-free Copy)
                    nc.scalar.activation(out=warm, in_=eps_sb, func=AF.Exp)

                # normalize q/k (x * rstd); v copy rides the Pool engine
                qn = p1.tile([128, 640], F32, tag="qn")
                for s in range(5):
                    nc.scalar.activation(out=qn[:, s * 128:(s + 1) * 128],
                                         in_=qkv_ps[:, s * 128:(s + 1) * 128],
                                         func=AF.Copy, scale=rstd[:, s:s + 1], bias=0.0)
                nc.vector.tensor_copy(vbuf[:, tt, :], qkv_ps[:, 640:768])

                # RoPE (norm weights folded into trig tables host-side)
                qr = p1.tile([128, 640], DT, tag="qr")
                qrs[tt] = qr

                def rope(seg0, nseg, toff):
                    src = qn[:, seg0 * 128:(seg0 + nseg) * 128]
                    dst = qr[:, seg0 * 128:(seg0 + nseg) * 128]
                    sev = src.rearrange("p (h j t) -> p h t j", t=2, j=64)
                    dev = dst.rearrange("p (h j t) -> p h t j", t=2, j=64)
                    qe, qo = sev[:, :, 0, :], sev[:, :, 1, :]
                    re, ro = dev[:, :, 0, :], dev[:, :, 1, :]
                    ce = bc4(trig_sb[:, toff + 0, :], nseg)
                    so = bc4(trig_sb[:, toff + 1, :], nseg)
                    se = bc4(trig_sb[:, toff + 2, :], nseg)
                    co = bc4(trig_sb[:, toff + 3, :], nseg)
                    ta = p1s.tile([128, nseg, 64], F32, tag=f"ra{toff}")
                    tb = p1s.tile([128, nseg, 64], F32, tag=f"rb{toff}")
                    nc.vector.tensor_mul(ta, qe, ce)
                    nc.gpsimd.tensor_mul(tb, qo, so)
                    nc.vector.tensor_sub(re, ta, tb)
                    tc_ = p1s.tile([128, nseg, 64], F32, tag=f"rc{toff}")
                    td = p1s.tile([128, nseg, 64], F32, tag=f"rd{toff}")
                    nc.gpsimd.tensor_mul(tc_, qe, se)
                    nc.vector.tensor_mul(td, qo, co)
                    nc.gpsimd.tensor_add(ro, tc_, td)

                rope(0, 4, 0)   # q heads, tables 0..3
                rope(4, 1, 4)   # k, tables 4..7

            def emit_scores(h, q0, j, scpool, sctag, ptpool):
                qlo = max(q0, j * 128)
                w = q0 + 512 - qlo
                sc = scpool.tile(*(([128, 768], F32) if sctag == "qkv"
                                   else ([128, 512], F32)), tag=sctag)
                nc.tensor.matmul(sc[:, 0:w], qkT[:, 4, j * 128:(j + 1) * 128],
                                 qkT[:, h, qlo:q0 + 512],
                                 start=True, stop=True)
                if qlo == j * 128:
                    nc.vector.tensor_add(sc[:, 0:128], sc[:, 0:128], maskt_sb)
                pT = ptpool.tile([128, 512], DT, tag="pT")
                nc.scalar.activation(out=pT[:, 0:w], in_=sc[:, 0:w],
                                     func=AF.Exp, scale=SCALE)
                return pT, qlo, w

            # seam pipeline: the first two attention rounds' scores+exp
            # (borrowing the qkv PSUM ring for score tiles) keep the PE
            # busy while the last two tiles' rope chains drain; then the
            # PE transposes for those tiles run with no stall, and the
            # deferred PV/sums follow once p2acc opens.
            deferred = {h: [emit_scores(h, 0, j, p1ps, "qkv", p2d)
                            for j in range(4)]
                        for h in (0, 1)}
            transpose_out(TB - 2)
            transpose_out(TB - 1)

        # ---------------- P2: attention (transposed scores) -------------
        with tc.tile_pool(name="p2acc", bufs=2, space="PSUM") as p2acc, \
             tc.tile_pool(name="p2sc", bufs=4, space="PSUM") as p2sc:
            def pv_sums(outT, sums, q0, jmax, j, pT, qlo, w):
                o0 = qlo - q0
                nc.tensor.matmul(outT[:, o0:512],
                                 vbuf[:, j, :], pT[:, 0:w],
                                 start=(j == 0), stop=(j == jmax),
                                 skip_group_check=True)
                nc.tensor.matmul(sums[:, o0:512],
                                 ones, pT[:, 0:w],
                                 start=(j == 0), stop=(j == jmax),
                                 skip_group_check=True)

            def finish(h, q0, outT, sums):
                inv = p2.tile([128, 512], F32, tag="inv")
                nc.vector.reciprocal_approx_fast(out=inv, in_=sums)
                nc.vector.tensor_mul(attT[:, h, q0:q0 + 512], outT, inv)

            for h in (0, 1):
                outT = p2acc.tile([128, 512], F32, tag="outT")
                sums = p2acc.tile([128, 512], F32, tag="sums")
                for j in range(4):
                    pT, qlo, w = deferred[h][j]
                    pv_sums(outT, sums, 0, 3, j, pT, qlo, w)
                finish(h, 0, outT, sums)
            for h in range(HQ):
                for qc in range(QC):
                    if qc == 0 and h < 2:
                        continue
                    q0 = qc * 512
                    jmax = 4 * qc + 3
                    outT = p2acc.tile([128, 512], F32, tag="outT")
                    sums = p2acc.tile([128, 512], F32, tag="sums")
                    for j in range(jmax + 1):
                        pT, qlo, w = emit_scores(h, q0, j, p2sc, "sc", p2)
                        pv_sums(outT, sums, q0, jmax, j, pT, qlo, w)
                    finish(h, q0, outT, sums)

        # ---------------- P3: output projection --------------------------
        with tc.tile_pool(name="p3ps", bufs=2, space="PSUM") as p3ps:
            for tt in range(TB):
                y_ps = p3ps.tile([128, 2048], F32, tag="y")
                for h in range(HQ):
                    for c4 in range(4):
                        nc.tensor.matmul(y_ps[:, c4 * 512:(c4 + 1) * 512],
                                         attT[:, h, tt * 128:(tt + 1) * 128],
                                         wo_sb[:, h, c4 * 512:(c4 + 1) * 512],
                                         start=(h == 0), stop=(h == HQ - 1))
                y_sb = p3.tile([128, 2048], DT, tag="ysb")
                nc.scalar.activation(out=y_sb[:, 0:512], in_=y_ps[:, 0:512],
                                     func=AF.Copy, scale=1.0, bias=0.0)
                nc.vector.tensor_copy(y_sb[:, 512:2048], y_ps[:, 512:2048])
                nc.sync.dma_start(out=y[tt * 128:(tt + 1) * 128, :], in_=y_sb)

    nc.compile()
    return nc


def _np_dt(dt_name):
    if dt_name == "bfloat16":
        import ml_dtypes
        return np.dtype(ml_dtypes.bfloat16)
    return np.dtype(np.float32)


def _prep_core_inputs(b, g, x, Wq, bq, Wk, bk, Wv, bv, Wo, bo, qn_w, kn_w,
                      freqs_cos, freqs_sin, mask, dt=np.float32):
    f32 = np.float32
    xb = np.ascontiguousarray(x[b], dtype=f32)
    # [tt, csub, cc, tcol]: xt[tt][p][cc*128+tc] = x[b][tt*128+tc][cc*128+p]
    xt = np.ascontiguousarray(
        xb.reshape(TB, 128, CB, 128).transpose(0, 3, 2, 1)
    ).reshape(TB, 128, CB * 128)
    wqkv = np.ascontiguousarray(np.concatenate([
        Wq[:, g * 512:(g + 1) * 512],
        Wk[:, g * 128:(g + 1) * 128],
        Wv[:, g * 128:(g + 1) * 128],
    ], axis=1).reshape(CB, 128, 768), dtype=f32)
    bqkv = np.concatenate([
        bq[g * 512:(g + 1) * 512], bk[g * 128:(g + 1) * 128],
        bv[g * 128:(g + 1) * 128],
    ]).reshape(1, 768).astype(f32)
    cos = freqs_cos.astype(f32)
    sin = freqs_sin.astype(f32)
    qe, qo = qn_w[0::2].astype(f32), qn_w[1::2].astype(f32)
    ke, ko = kn_w[0::2].astype(f32), kn_w[1::2].astype(f32)
    # tables: [ce, so, se, co] for q then for k; layout [TB, 128, 8*64]
    tabs = np.stack([cos * qe, sin * qo, sin * qe, cos * qo,
                     cos * ke, sin * ko, sin * ke, cos * ko], axis=1)  # [T, 8, 64]
    trig = np.ascontiguousarray(tabs.reshape(TB, 128, 8 * 64), dtype=f32)
    maskt = np.ascontiguousarray(mask[0, 0, :128, :128].T, dtype=f32)
    cst = np.stack([np.ones((128, 128), f32), np.eye(128, dtype=f32)])
    wo_t = np.ascontiguousarray(
        Wo[g * 512:(g + 1) * 512].reshape(HQ, 128, C), dtype=f32)
    out = {"xt": xt, "wqkv": wqkv, "bqkv": bqkv, "trig": trig,
           "maskt": maskt, "cst": cst, "wo": wo_t}
    if np.dtype(dt) != np.float32:
        out = {k: (v if k == "maskt" else np.ascontiguousarray(v.astype(dt)))
               for k, v in out.items()}
    return out


def kernel(x, Wq, bq, Wk, bk, Wv, bv, Wo, bo, qn_w, kn_w,
           freqs_cos, freqs_sin, mask, _trace=False, _trace_kwargs=None):
    from concourse.bass_utils import run_bass_kernel_spmd

    args = (np.asarray(x), np.asarray(Wq), np.asarray(bq), np.asarray(Wk),
            np.asarray(bk), np.asarray(Wv), np.asarray(bv), np.asarray(Wo),
            np.asarray(bo), np.asarray(qn_w), np.asarray(kn_w),
            np.asarray(freqs_cos), np.asarray(freqs_sin), np.asarray(mask))
    bo_np = args[8].astype(np.float32)
    use_bias = bool(np.any(args[2]) or np.any(args[4]) or np.any(args[6]))

    dt_name = os.environ.get("BASS_ATTN_DT", "bfloat16")
    key = (dt_name, use_bias)
    if key not in _CACHE:
        _CACHE[key] = _build(dt_name, use_bias)
    nc = _CACHE[key]

    np_dt = _np_dt(dt_name)
    in_maps = [_prep_core_inputs(cid // 4, cid % 4, *args, dt=np_dt)
               for cid in range(8)]
    res = run_bass_kernel_spmd(nc, in_maps, core_ids=list(range(8)),
                               trace=_trace, **(_trace_kwargs or {}))
    outs = [np.asarray(res.results[i]["y"], dtype=np.float32) for i in range(8)]
    yfull = np.empty((B, T, C), dtype=np.float32)
    for b in range(B):
        yfull[b] = outs[4 * b] + outs[4 * b + 1] + outs[4 * b + 2] + outs[4 * b + 3]
        yfull[b] += bo_np[None, :]
    if _trace:
        kernel._last_result = res
    return yfull
